# revision 6
# baseline (speedup 1.0000x reference)
"""DeltaRule (order-1 / transition) forward as a Trainium2 Bass kernel.

Math (per sequence, binary obs x_t, obs_prev x_{t-1}, eff_lr = clip(lr,0,1)):
    p0_t = p0' + lr*(x_t - p0')*(1 - x_{t-1})
    p1_t = p1' + lr*(x_t - p1')*x_{t-1}
    pred_t = p0_t*(1-x_t) + p1_t*x_t ,  p0_0' = p1_0' = 0.5, x_{-1} = 0

Rewritten as two first-order linear recurrences (scaled by 1/lr so the
inhomogeneous terms are exactly-representable {0,1}):
    r0_t = a0_t * r0_{t-1} + b0_t   a0 = 1 - s*(1-xp), b0 = x*(1-xp) = x - x*xp
    r1_t = a1_t * r1_{t-1} + b1_t   a1 = 1 - s*xp,     b1 = x*xp
    r*_init = 0.5/lr,  p* = lr * r*,  pred = lr * (x ? r1 : r0)
with s = 1 - fl32(1-lr)  (so the gated-off branch coefficient is EXACTLY 1.0:
a = fl(beta + s*1) = fl(beta + (1-beta)) = 1 by Sterbenz).

Device layout: [seq on partitions, time on free dim]; the sequential scan runs
on the Vector engine via tensor_tensor_scan (fp32 internal state).

I/O transport is the measured bottleneck (the per-execute cost through the
axon-tunneled PJRT scales ~26-32 us per MB of operand/result bytes, plus a
~3.4 ms fixed dispatch floor that grows ~0.3 ms/core):
  - input x is bit-packed host-side to 1 bit/obs (uint8 [B_C, T/8], bit k of
    byte j = x[t = k*1024 + j]); the device unpacks with one fused
    shift-and tensor_scalar per 1024-column slab.
  - output pred is quantized on-device to uint8 (q = 255*lr*r + 0.5,
    truncating convert) when lr is small enough that pred provably stays
    well away from 0 (max quantization rel-err ~0.6% vs the 2e-2 gate);
    bf16 otherwise. The host dequantizes as part of unsharding.
  - the zero ExternalOutput operand is donated (as in the canonical
    run_bass_via_pjrt), which measurably removes its per-call staging cost;
    the bench ping-pongs each call's result back in as the next donated
    output buffer.
  - 2 cores, not 8: pure data-parallel over sequences; per-core device time
    (~0.3 ms) is far below the per-core dispatch cost of extra cores.
"""

import os
import sys

import numpy as np

for _p in ("/opt/trn_rl_repo", "/root/.axon_site/_ro/trn_rl_repo"):
    if os.path.isdir(_p) and _p not in sys.path:
        sys.path.insert(0, _p)

import concourse.bass as bass
import concourse.bacc as bacc
import concourse.mybir as mybir
import concourse.tile as tile
from concourse import bass2jax

F32 = mybir.dt.float32
BF16 = mybir.dt.bfloat16
U8 = mybir.dt.uint8
Alu = mybir.AluOpType
Copy = mybir.ActivationFunctionType.Copy

T = 8192          # n_time_steps
B = 4096          # n_seqs (full)

# Knobs (env overrides are for local experiments only; defaults are the
# shipping configuration)
N_CORES = int(os.environ.get("DR_CORES", "2"))
B_C = B // N_CORES
CHUNK = 2048      # time-chunk per scan instruction
PACK_BITS = os.environ.get("DR_PACK", "1") == "1"
OUT_U8_MAX_LR = float(os.environ.get("DR_U8_MAX_LR", "0.02"))
DONATE = os.environ.get("DR_DONATE", "1") == "1"
# f32->u8 conversion on the ACT engine rounds to nearest (measured: bias 0.5
# doubles the quantization error), so no rounding bias is needed.
QBIAS = float(os.environ.get("DR_QBIAS", "0.0"))

LAST_RESULTS = None  # list[dict[name, np.ndarray]] of the most recent run
LAST_BENCH = None    # (step, block) closures for timing


def _build_nc(eff_lr: float, out_u8: bool, b_c: int = B_C, t_len: int = T,
              ch: int = CHUNK):
    """Build the single-core Bass program (SPMD: identical on all cores)."""
    beta = float(np.float32(1.0) - np.float32(eff_lr))  # fl32(1-lr)
    s = 1.0 - beta  # exact in f32; |s - lr| <= 1 ulp
    rinit = float(np.float32(0.5) / np.float32(eff_lr))
    lr32 = float(np.float32(eff_lr))
    n_seq_tiles = b_c // 128
    n_chunks = t_len // ch
    n_slabs = 8
    slab = t_len // n_slabs  # 1024

    nc = bacc.Bacc("TRN2", target_bir_lowering=False, debug=False)
    if PACK_BITS:
        xt = nc.dram_tensor(
            "xt", [b_c, t_len // 8], U8, kind="ExternalInput"
        ).ap()
    else:
        xt = nc.dram_tensor("xt", [b_c, t_len], U8, kind="ExternalInput").ap()
    out_dt = U8 if out_u8 else BF16
    pred = nc.dram_tensor("pred", [b_c, t_len], out_dt, kind="ExternalOutput").ap()

    with tile.TileContext(nc) as tc:
        with (
            tc.tile_pool(name="xp8", bufs=2) as ppool,
            tc.tile_pool(name="xb", bufs=2) as xpool,
            tc.tile_pool(name="coef", bufs=2) as cpool,
            tc.tile_pool(name="bb", bufs=4) as bpool,
            tc.tile_pool(name="r0", bufs=3) as r0pool,
            tc.tile_pool(name="r1", bufs=3) as r1pool,
            tc.tile_pool(name="qq", bufs=2) as qpool,
        ):
            for si in range(n_seq_tiles):
                rows = slice(si * 128, (si + 1) * 128)

                # x as uint8 {0,1} on SBUF; column 0 holds the x_{t-1}=0
                # boundary, data in columns 1..T. (bitVec TSP ops cannot
                # cast, so the unpack stays u8 -> u8; downstream arithmetic
                # ops read u8 and cast on write where needed.)
                xbig = xpool.tile([128, t_len + 1], U8, tag="xb")
                nc.vector.memset(xbig[:, 0:1], 0)
                if PACK_BITS:
                    xpt = ppool.tile([128, t_len // 8], U8, tag="xp8")
                    nc.gpsimd.dma_start(xpt[:], xt[rows, :])
                    for k in range(n_slabs):
                        # slab k covers t in [k*1024, (k+1)*1024):
                        # (byte >> k) & 1  (bitVec ops: DVE only, Pool lacks them)
                        nc.vector.tensor_scalar(
                            xbig[:, 1 + k * slab : 1 + (k + 1) * slab],
                            xpt[:],
                            k,
                            1,
                            Alu.logical_shift_right,
                            Alu.bitwise_and,
                        )
                else:
                    nc.gpsimd.dma_start(xbig[:, 1 : t_len + 1], xt[rows, :])

                # full-row output staging tile; one store per seq-tile
                qbig = qpool.tile([128, t_len], out_dt, tag="qq")

                prev_r0 = prev_r1 = None
                pend = None  # deferred (r0, r1, xc, k)

                def emit_out(r0_, r1_, xc_, k_):
                    # mask is integer-typed u8 {0,1}: nonzero-means-copy.
                    # r0 <- r1 where x==1  (pred selector)
                    nc.vector.copy_predicated(r0_, xc_, r1_)
                    dst = qbig[:, k_ * ch : (k_ + 1) * ch]
                    if out_u8:
                        # q = trunc(255*lr*r + 0.5) -> uint8
                        nc.scalar.activation(
                            dst, r0_, Copy, bias=QBIAS, scale=255.0 * lr32
                        )
                    else:
                        # pred = lr*r -> bf16
                        nc.scalar.activation(dst, r0_, Copy, bias=0.0, scale=lr32)

                for k in range(n_chunks):
                    xp = xbig[:, k * ch : (k + 1) * ch]          # x_{t-1}
                    xc = xbig[:, k * ch + 1 : (k + 1) * ch + 1]  # x_t

                    # coefficients (ScalarE): a0 = beta + s*xp, a1 = 1 - s*xp
                    a0 = cpool.tile([128, ch], F32, tag="a0")
                    a1 = cpool.tile([128, ch], F32, tag="a1")
                    nc.scalar.activation(a0[:], xp, Copy, bias=beta, scale=s)
                    nc.scalar.activation(a1[:], xp, Copy, bias=1.0, scale=-s)

                    # inhomogeneous terms, both direct from x (no serial dep):
                    # b1 = xx = x*xp on GpSimd, b0 = (x > xp) on VectorE
                    xx = bpool.tile([128, ch], BF16, tag="xx")
                    b0 = bpool.tile([128, ch], BF16, tag="b0")
                    nc.gpsimd.tensor_tensor(xx[:], xc, xp, Alu.mult)
                    nc.vector.tensor_tensor(b0[:], xc, xp, Alu.is_gt)

                    r0 = r0pool.tile([128, ch], F32, tag="r0")
                    r1 = r1pool.tile([128, ch], F32, tag="r1")
                    i0 = rinit if k == 0 else prev_r0[:, ch - 1 : ch]
                    i1 = rinit if k == 0 else prev_r1[:, ch - 1 : ch]
                    nc.vector.tensor_tensor_scan(
                        r0[:], a0[:], b0[:], i0, Alu.mult, Alu.add
                    )
                    nc.vector.tensor_tensor_scan(
                        r1[:], a1[:], xx[:], i1, Alu.mult, Alu.add
                    )

                    # predicate+quantize the previous chunk only after this
                    # chunk's scans have consumed prev_r0[:, -1]
                    # (copy_predicated overwrites it)
                    if pend is not None:
                        emit_out(*pend)
                    pend = (r0[:], r1[:], xc, k)
                    prev_r0, prev_r1 = r0, r1
                emit_out(*pend)
                nc.gpsimd.dma_start(pred[rows, :], qbig[:])
    nc.compile()
    return nc


def _run_spmd(nc, in_maps):
    """Mirror of bass2jax.run_bass_via_pjrt's multi-core branch, donating the
    zero ExternalOutput operands like the canonical path, and caching a
    ping-pong step closure so test.py can re-execute the NEFF for timing.
    Returns list[dict[name, np.ndarray]] per core."""
    global LAST_BENCH
    import jax
    from jax.sharding import Mesh, PartitionSpec
    from jax.experimental.shard_map import shard_map
    import concourse.mybir as _mybir

    bass2jax.install_neuronx_cc_hook()
    n_cores = len(in_maps)

    partition_name = (
        nc.partition_id_tensor.name if nc.partition_id_tensor else None
    )
    in_names, out_names, out_avals, zero_outs = [], [], [], []
    for alloc in nc.m.functions[0].allocations:
        if not isinstance(alloc, _mybir.MemoryLocationSet):
            continue
        name = alloc.memorylocations[0].name
        if alloc.kind == "ExternalInput":
            if name != partition_name:
                in_names.append(name)
        elif alloc.kind == "ExternalOutput":
            shape = tuple(alloc.tensor_shape)
            dtype = _mybir.dt.np(alloc.dtype)
            out_names.append(name)
            out_avals.append(jax.core.ShapedArray(shape, dtype))
            zero_outs.append(np.zeros(shape, dtype))
    n_params = len(in_names)
    n_outs = len(out_avals)
    all_names = in_names + out_names
    if partition_name is not None:
        all_names = all_names + [partition_name]

    def _body(*args):
        operands = list(args)
        if partition_name is not None:
            operands.append(bass2jax.partition_id_tensor())
        outs = bass2jax._bass_exec_p.bind(
            *operands,
            out_avals=tuple(out_avals),
            in_names=tuple(all_names),
            out_names=tuple(out_names),
            lowering_input_output_aliases=(),
            sim_require_finite=True,
            sim_require_nnan=True,
            nc=nc,
        )
        return tuple(outs)

    devices = jax.devices()[:n_cores]
    mesh = Mesh(np.asarray(devices), ("core",))
    in_specs = (PartitionSpec("core"),) * (n_params + n_outs)
    out_specs = (PartitionSpec("core"),) * n_outs
    donate = tuple(range(n_params, n_params + n_outs)) if DONATE else ()
    sharded = jax.jit(
        shard_map(
            _body, mesh=mesh, in_specs=in_specs, out_specs=out_specs,
            check_rep=False,
        ),
        donate_argnums=donate,
        keep_unused=True,
    )
    concat_in = [
        np.concatenate([np.asarray(m[name]) for m in in_maps], axis=0)
        for name in in_names
    ]
    concat_zeros = [
        np.zeros((n_cores * z.shape[0], *z.shape[1:]), z.dtype) for z in zero_outs
    ]
    in_args = [jax.device_put(a) for a in concat_in]
    outs = tuple(jax.device_put(z) for z in concat_zeros)
    outs = jax.block_until_ready(sharded(*in_args, *outs))

    state = {"outs": outs}

    def step():
        # each call donates the previous call's result buffers back in as
        # the output operands (their contents are fully overwritten)
        state["outs"] = sharded(*in_args, *state["outs"])
        return state["outs"]

    def block():
        state["outs"] = jax.block_until_ready(state["outs"])

    LAST_BENCH = (step, block)
    return [
        {
            name: np.asarray(outs[i]).reshape(n_cores, *out_avals[i].shape)[c]
            for i, name in enumerate(out_names)
        }
        for c in range(n_cores)
    ]


def bench_ns(iters: int = 20) -> float:
    """Per-execution wall time (ns) of the cached NEFF, amortized over iters."""
    import time as _time

    step, block = LAST_BENCH
    step()
    block()  # warm
    t0 = _time.perf_counter()
    for _ in range(iters):
        step()
    block()
    return (_time.perf_counter() - t0) / iters * 1e9


def _pack_inputs(x: np.ndarray):
    """(T,B,1) f32 -> per-core input maps."""
    xb = np.ascontiguousarray(x[:, :, 0].T != 0.0)  # (B, T) bool
    if PACK_BITS:
        # byte j, bit k  <->  t = k*(T/8) + j
        xr = xb.reshape(B, 8, T // 8)
        packed = np.packbits(xr, axis=1, bitorder="little")[:, 0, :]  # (B, T/8)
        full = np.ascontiguousarray(packed)
    else:
        full = np.ascontiguousarray(xb.view(np.uint8))
    return [
        {"xt": np.ascontiguousarray(full[c * B_C : (c + 1) * B_C])}
        for c in range(N_CORES)
    ]


def kernel(x: np.ndarray, lr: np.ndarray) -> np.ndarray:
    """Full (T,B,1) f32 in -> full (T,B,1) f32 out, computed on NeuronCores."""
    global LAST_RESULTS
    eff_lr = float(np.clip(np.float32(lr), 0.0, 1.0))
    x = np.asarray(x, dtype=np.float32)
    assert x.shape == (T, B, 1), x.shape
    if eff_lr == 0.0:
        # degenerate: state never updates; pred = 0.5 everywhere
        return np.full((T, B, 1), 0.5, np.float32)

    out_u8 = eff_lr <= OUT_U8_MAX_LR
    in_maps = _pack_inputs(x)

    # The axon terminal occasionally throws a transient
    # NRT_EXEC_UNIT_UNRECOVERABLE on the first execute; one rebuild+retry
    # has always recovered it.
    try:
        nc = _build_nc(eff_lr, out_u8)
        LAST_RESULTS = _run_spmd(nc, in_maps)
    except Exception:
        import time as _time

        _time.sleep(5.0)
        nc = _build_nc(eff_lr, out_u8)
        LAST_RESULTS = _run_spmd(nc, in_maps)

    preds = [LAST_RESULTS[c]["pred"] for c in range(N_CORES)]  # (B_C, T)
    full = np.concatenate([np.asarray(p) for p in preds], axis=0)  # (B, T)
    if out_u8:
        out = full.astype(np.float32) * np.float32(1.0 / 255.0)
    else:
        out = full.astype(np.float32)
    return np.ascontiguousarray(out.T)[:, :, None].astype(np.float32)
